# revision 1
# baseline (speedup 1.0000x reference)
"""Trainium2 Bass kernel for a dense transformer block (self-attn causal +
cross-attn + MLP), sharded over 8 NeuronCores without collectives.

Sharding: core c -> batch b = c//2, parity p = c%2. Each core computes the
output for query rows p::2 of batch b (1024 rows). K/V for self-attention are
recomputed per-core for the full 2048-row sequence (row-local ops are cheap
relative to collectives). The strided query split gives every core an
identical program structure; causality enters only through per-core mask
data (qg/kg index tensors).

Precision: fp32r (full-rate PE, ~1e-4 rel err) for all projections and
q.k scores; bf16 for attention-weights x V and both MLP matmuls; fp32
accumulation in PSUM; fp32 LayerNorm/softmax bookkeeping.
"""
import sys

sys.path.insert(0, "/opt/trn_rl_repo")

import numpy as np
import ml_dtypes

import concourse.bass as bass
import concourse.tile as tile
from concourse import bacc, mybir
from concourse.bass_utils import run_bass_kernel_spmd
from concourse.masks import make_identity

F32 = mybir.dt.float32
F32R = mybir.dt.float32r
BF16 = mybir.dt.bfloat16
AF = mybir.ActivationFunctionType
OP = mybir.AluOpType

B, T, S, D = 4, 2048, 512, 768
NINP = 768
PROT = 1024
H, HD, HID = 12, 64, 3072
TQ = T // 2            # own query rows per core
DC = D // 128          # 6 feature chunks
EC = PROT // 128       # 8 encoder feature chunks
HCN = HID // 128       # 24 hidden chunks
NTT = T // 128         # 16 token tiles (full seq)
NQT = TQ // 128        # 8 own token tiles
EPS = 1e-5

_CACHE: dict = {}


def _bcast_ap(handle, offset, nfree):
    t = getattr(handle, "tensor", handle)
    return bass.AP(tensor=t, offset=offset, ap=[[0, 128], [1, nfree]])


def _build():
    nc = bacc.Bacc("TRN2", target_bir_lowering=False, debug=False)

    # ---- DRAM I/O ----
    x_full = nc.dram_tensor("x_full", [T, D], F32, kind="ExternalInput")
    x_own = nc.dram_tensor("x_own", [TQ, D], F32, kind="ExternalInput")
    enc_t = nc.dram_tensor("enc_t", [PROT, S], F32R, kind="ExternalInput")
    wq = nc.dram_tensor("wq", [D, D], F32R, kind="ExternalInput")   # pre-scaled
    wk = nc.dram_tensor("wk", [D, D], F32R, kind="ExternalInput")
    wv = nc.dram_tensor("wv", [D, D], F32R, kind="ExternalInput")
    wo = nc.dram_tensor("wo", [D, D], F32R, kind="ExternalInput")
    cwq = nc.dram_tensor("cwq", [D, D], F32R, kind="ExternalInput")  # pre-scaled
    cwk = nc.dram_tensor("cwk", [PROT, D], F32R, kind="ExternalInput")
    cwv = nc.dram_tensor("cwv", [PROT, D], F32R, kind="ExternalInput")
    cwo = nc.dram_tensor("cwo", [D, D], F32R, kind="ExternalInput")
    mw1 = nc.dram_tensor("mw1", [D, HID], BF16, kind="ExternalInput")
    mw2 = nc.dram_tensor("mw2", [HID, D], BF16, kind="ExternalInput")
    bq = nc.dram_tensor("bq", [D], F32, kind="ExternalInput")        # pre-scaled
    bk = nc.dram_tensor("bk", [D], F32, kind="ExternalInput")
    bv = nc.dram_tensor("bv", [D], F32, kind="ExternalInput")
    bo = nc.dram_tensor("bo", [D], F32, kind="ExternalInput")
    cbq = nc.dram_tensor("cbq", [D], F32, kind="ExternalInput")      # pre-scaled
    cbk = nc.dram_tensor("cbk", [D], F32, kind="ExternalInput")
    cbv = nc.dram_tensor("cbv", [D], F32, kind="ExternalInput")
    cbo = nc.dram_tensor("cbo", [D], F32, kind="ExternalInput")
    mb1 = nc.dram_tensor("mb1", [HID], F32, kind="ExternalInput")
    mb2 = nc.dram_tensor("mb2", [D], F32, kind="ExternalInput")
    ln1g = nc.dram_tensor("ln1g", [D], F32, kind="ExternalInput")
    ln1b = nc.dram_tensor("ln1b", [D], F32, kind="ExternalInput")
    qg_row = nc.dram_tensor("qg_row", [1, TQ], F32, kind="ExternalInput")
    kg_cols = nc.dram_tensor("kg_cols", [128, NTT], F32, kind="ExternalInput")
    out_own = nc.dram_tensor("out_own", [TQ, D], F32, kind="ExternalOutput")

    with tile.TileContext(nc) as tc:
        # ---- pool stack (release order = reverse alloc order) ----
        singles = tc.alloc_tile_pool(name="singles", bufs=1)
        dram = tc.alloc_tile_pool(name="dram", bufs=1, space="DRAM")
        pq = tc.alloc_tile_pool(name="pq", bufs=1)
        pk = tc.alloc_tile_pool(name="pk", bufs=1)
        pv = tc.alloc_tile_pool(name="pv", bufs=1)

        ident = singles.tile([128, 128], F32, name="ident")
        make_identity(nc, ident[:, :])
        eps_t = singles.tile([128, 1], F32, name="eps")
        nc.vector.memset(eps_t, EPS)
        ones_f = singles.tile([128, 128], F32, name="ones_f")
        nc.vector.memset(ones_f, 1.0)
        ones_r1 = singles.tile([1, 128], F32R, name="ones_r1")
        nc.vector.tensor_copy(ones_r1[0:1, :], ones_f[0:1, :])
        ones12 = singles.tile([128, 12], F32, name="ones12")
        nc.vector.memset(ones12, 1.0)
        kg_sb = singles.tile([128, NTT], F32, name="kg_sb")
        nc.sync.dma_start(out=kg_sb, in_=kg_cols[:, :])

        def bias6(h, name, pool=None):  # [768] -> [128, 6] per-partition
            t = (pool or singles).tile([128, DC], F32, name=name)
            nc.sync.dma_start(out=t, in_=h.ap().rearrange("(c p) -> p c", p=128))
            return t

        def bias_bc(h, name, pool, n=D):  # [n] -> [128, n] bcast
            t = pool.tile([128, n], F32, name=name)
            nc.gpsimd.dma_start(out=t, in_=_bcast_ap(h, 0, n))
            return t

        bq6 = bias6(bq, "bq6")
        bk6 = bias6(bk, "bk6")
        cbq6 = bias6(cbq, "cbq6")
        cbk6 = bias6(cbk, "cbk6")
        l1g_bc = bias_bc(ln1g, "l1g_bc", singles)
        l1b_bc = bias_bc(ln1b, "l1b_bc", singles)

        xn_own_sp = dram.tile([TQ, D], F32, name="xn_own_sp")
        x1_sp = dram.tile([TQ, D], F32, name="x1_sp")
        x2_sp = dram.tile([TQ, D], F32, name="x2_sp")

        q_fm = [pq.tile([128, TQ], F32R, name=f"qfm{dc}") for dc in range(DC)]
        k_fm = [pk.tile([128, T], F32R, name=f"kfm{dc}") for dc in range(DC)]
        v_tok = [pv.tile([128, H, HD + 1], BF16, name=f"vtok{tt}")
                 for tt in range(NTT)]

        def ln_tile(spool, xt, out):
            xr = xt.rearrange("p (s f) -> p s f", f=256)
            stats = spool.tile([128, 3, 6], F32, name="bnst")
            for s in range(3):
                nc.vector.bn_stats(out=stats[:, s, :], in_=xr[:, s, :])
            mv = spool.tile([128, 2], F32, name="bnmv")
            nc.vector.bn_aggr(out=mv, in_=stats)
            std = spool.tile([128, 1], F32, name="bnstd")
            nc.scalar.activation(std, mv[:, 1:2], AF.Sqrt, bias=eps_t)
            rstd = spool.tile([128, 1], F32, name="bnrstd")
            nc.vector.reciprocal(rstd, std)
            nc.vector.tensor_scalar(out, xt, mv[:, 0:1], rstd,
                                    OP.subtract, OP.mult)

        # ===== Phase A: own rows: LN1 -> spill + Q projection =============
        with tc.tile_pool(name="wqp", bufs=1) as wqp, \
             tc.tile_pool(name="pA", bufs=4) as pA, \
             tc.tile_pool(name="pAs", bufs=6) as pAs, \
             tc.tile_pool(name="pAfm", bufs=2) as pAfm, \
             tc.tile_pool(name="pAps", bufs=4, space="PSUM") as pAps, \
             tc.tile_pool(name="pAmm", bufs=2, space="PSUM") as pAmm:
            xno_blks = []
            for tb in range(TQ // 512):
                xno_blk = [pAfm.tile([128, 512], F32R, name=f"xnoblk{dc}")
                           for dc in range(DC)]
                xno_blks.append(xno_blk)
                for t4 in range(4):
                    tt = tb * 4 + t4
                    xt = pA.tile([128, D], F32, name="xt")
                    nc.sync.dma_start(
                        out=xt, in_=x_own[tt * 128:(tt + 1) * 128, :])
                    xnt = pA.tile([128, D], F32, name="xnt")
                    ln_tile(pAs, xt, xnt)
                    xgb = pA.tile([128, D], F32, name="xgb")
                    nc.vector.tensor_mul(xgb, xnt, l1g_bc)
                    nc.vector.tensor_add(xgb, xgb, l1b_bc)
                    nc.sync.dma_start(
                        out=xn_own_sp[tt * 128:(tt + 1) * 128, :], in_=xgb)
                    for dc in range(DC):
                        pt = pAps.tile([128, 128], F32, name="trpA")
                        nc.tensor.transpose(
                            pt, xnt[:, dc * 128:(dc + 1) * 128], ident)
                        nc.scalar.copy(
                            xno_blk[dc][:, t4 * 128:(t4 + 1) * 128], pt)
            wq_sb = [wqp.tile([128, D], F32R, name=f"wq{dk}")
                     for dk in range(DC)]
            for dk in range(DC):
                nc.sync.dma_start(out=wq_sb[dk],
                                  in_=wq[dk * 128:(dk + 1) * 128, :])
            for tb in range(TQ // 512):
                xno_blk = xno_blks[tb]
                for dc in range(DC):
                    pp = pAmm.tile([128, 512], F32, name="qpp")
                    for dk in range(DC):
                        nc.tensor.matmul(
                            pp, wq_sb[dk][:, dc * 128:(dc + 1) * 128],
                            xno_blk[dk], start=(dk == 0), stop=(dk == DC - 1))
                    nc.scalar.activation(
                        q_fm[dc][:, tb * 512:(tb + 1) * 512], pp,
                        AF.Identity, bias=bq6[:, dc:dc + 1])

        # ===== Phase B: full seq: LN1 -> K and V projections ==============
        with tc.tile_pool(name="wkv", bufs=1) as wkv, \
             tc.tile_pool(name="pB", bufs=4) as pB, \
             tc.tile_pool(name="pBs", bufs=6) as pBs, \
             tc.tile_pool(name="pBfm", bufs=2) as pBfm, \
             tc.tile_pool(name="pBps", bufs=4, space="PSUM") as pBps, \
             tc.tile_pool(name="pBmm", bufs=2, space="PSUM") as pBmm:
            wk_sb = [wkv.tile([128, D], F32R, name=f"wk{dk}")
                     for dk in range(DC)]
            wv_sb = [wkv.tile([128, D], F32R, name=f"wv{dk}")
                     for dk in range(DC)]
            for dk in range(DC):
                nc.sync.dma_start(out=wk_sb[dk],
                                  in_=wk[dk * 128:(dk + 1) * 128, :])
                nc.sync.dma_start(out=wv_sb[dk],
                                  in_=wv[dk * 128:(dk + 1) * 128, :])
            bv_bc = bias_bc(bv, "bv_bc", wkv)
            for tb in range(T // 512):
                xn_blk = [pBfm.tile([128, 512], F32R, name=f"xnblk{dc}")
                          for dc in range(DC)]
                for t4 in range(4):
                    tt = tb * 4 + t4
                    xt = pB.tile([128, D], F32, name="xtB")
                    nc.sync.dma_start(
                        out=xt, in_=x_full[tt * 128:(tt + 1) * 128, :])
                    xnt = pB.tile([128, D], F32, name="xntB")
                    ln_tile(pBs, xt, xnt)
                    for dc in range(DC):
                        pt = pBps.tile([128, 128], F32, name="trpB")
                        nc.tensor.transpose(
                            pt, xnt[:, dc * 128:(dc + 1) * 128], ident)
                        nc.scalar.copy(
                            xn_blk[dc][:, t4 * 128:(t4 + 1) * 128], pt)
                for dc in range(DC):
                    pp = pBmm.tile([128, 512], F32, name="kpp")
                    for dk in range(DC):
                        nc.tensor.matmul(
                            pp, wk_sb[dk][:, dc * 128:(dc + 1) * 128],
                            xn_blk[dk], start=(dk == 0), stop=(dk == DC - 1))
                    nc.scalar.activation(
                        k_fm[dc][:, tb * 512:(tb + 1) * 512], pp,
                        AF.Identity, bias=bk6[:, dc:dc + 1])
                for t4 in range(4):
                    tt = tb * 4 + t4
                    vt = v_tok[tt]
                    for hf in range(2):
                        pp = pBmm.tile([128, 384], F32, name="vpp")
                        for dk in range(DC):
                            nc.tensor.matmul(
                                pp,
                                xn_blk[dk][:, t4 * 128:(t4 + 1) * 128],
                                wv_sb[dk][:, hf * 384:(hf + 1) * 384],
                                start=(dk == 0), stop=(dk == DC - 1))
                        nc.vector.tensor_add(
                            vt[:, hf * 6:(hf + 1) * 6, 0:HD], pp,
                            bv_bc[:, hf * 384:(hf + 1) * 384])
                    nc.vector.tensor_copy(vt[:, :, HD:HD + 1], ones12)

        # ===== Phase 3: causal self-attention =============================
        with tc.tile_pool(name="wop", bufs=1) as wop, \
             tc.tile_pool(name="yp", bufs=4) as yp, \
             tc.tile_pool(name="selp", bufs=2) as selp, \
             tc.tile_pool(name="pp3", bufs=4) as pp3, \
             tc.tile_pool(name="pf3", bufs=2) as pf3, \
             tc.tile_pool(name="r3", bufs=2) as r3, \
             tc.tile_pool(name="ysb3", bufs=2) as ysb3, \
             tc.tile_pool(name="x13", bufs=2) as x13, \
             tc.tile_pool(name="sps3", bufs=2, space="PSUM") as sps3, \
             tc.tile_pool(name="yps3", bufs=2, space="PSUM") as yps3, \
             tc.tile_pool(name="rb3", bufs=1, space="PSUM") as rb3, \
             tc.tile_pool(name="ops3", bufs=1, space="PSUM") as ops3:
            wo_sb = [wop.tile([128, D], F32R, name=f"wo{dk}")
                     for dk in range(DC)]
            for dk in range(DC):
                nc.sync.dma_start(out=wo_sb[dk],
                                  in_=wo[dk * 128:(dk + 1) * 128, :])
            bo_bc = bias_bc(bo, "bo_bc", wop)
            for qb in range(4):
                nch = 4 * (qb + 1)
                qgb_t = selp.tile([128, 256], F32, name="qgb")
                nc.gpsimd.dma_start(out=qgb_t,
                                    in_=_bcast_ap(qg_row, qb * 256, 256))
                selq = selp.tile([128, 1024], F32, name="selq")
                for cr in range(4):
                    c = nch - 4 + cr
                    nc.gpsimd.tensor_scalar(
                        selq[:, cr * 256:(cr + 1) * 256], qgb_t,
                        kg_sb[:, c:c + 1], None, OP.is_ge)
                yts = [yp.tile([128, 256], F32R, name=f"Y{dk}")
                       for dk in range(DC)]
                ng = nch // 4
                for h in range(H):
                    kb, ko = h // 2, (h % 2) * 64
                    y_ps = yps3.tile([128, 256], F32, name="yps")
                    for g in range(ng):
                        c0 = 4 * g
                        sps_t = sps3.tile([128, 1024], F32, name="sps")
                        for j in range(4):
                            c = c0 + j
                            nc.tensor.matmul(
                                sps_t[:, j * 256:(j + 1) * 256],
                                k_fm[kb][ko:ko + 64, c * 128:(c + 1) * 128],
                                q_fm[kb][ko:ko + 64,
                                         qb * 256:(qb + 1) * 256],
                                start=True, stop=True)
                        p_t = pp3.tile([128, 1024], BF16, name="P")
                        if g == ng - 1:
                            pf_t = pf3.tile([128, 1024], F32, name="Pf")
                            nc.scalar.activation(pf_t, sps_t, AF.Exp)
                            nc.vector.tensor_mul(p_t, pf_t, selq)
                        else:
                            nc.scalar.activation(p_t, sps_t, AF.Exp)
                        for j in range(4):
                            c = c0 + j
                            nc.tensor.matmul(
                                y_ps[0:HD + 1, :], v_tok[c][:, h, :],
                                p_t[:, j * 256:(j + 1) * 256],
                                start=(c == 0), stop=(c == nch - 1))
                    rd = r3.tile([1, 256], F32R, name="rr")
                    with nc.allow_low_precision(reason="softmax denom"):
                        nc.vector.reciprocal(rd[0:1, :], y_ps[64:65, :])
                    rb_ps = rb3.tile([128, 256], F32, name="rb")
                    nc.tensor.matmul(rb_ps, ones_r1[0:1, :], rd[0:1, :],
                                     start=True, stop=True)
                    y_sb = ysb3.tile([64, 256], F32, name="ysb")
                    nc.vector.tensor_copy(y_sb, y_ps[0:64, :])
                    nc.vector.tensor_mul(yts[kb][ko:ko + 64, :], y_sb,
                                         rb_ps[0:64, :])
                for tch in range(2):
                    ttg = qb * 2 + tch
                    xn_rb = x13.tile([128, D], F32, name="xnrb")
                    nc.sync.dma_start(
                        out=xn_rb,
                        in_=xn_own_sp[ttg * 128:(ttg + 1) * 128, :])
                    x1_t = x13.tile([128, D], F32, name="x1t")
                    for hf in range(2):
                        op_ps = ops3.tile([128, 384], F32, name="ops")
                        for dk in range(DC):
                            nc.tensor.matmul(
                                op_ps,
                                yts[dk][:, tch * 128:(tch + 1) * 128],
                                wo_sb[dk][:, hf * 384:(hf + 1) * 384],
                                start=(dk == 0), stop=(dk == DC - 1))
                        nc.vector.tensor_add(
                            x1_t[:, hf * 384:(hf + 1) * 384], op_ps,
                            bo_bc[:, hf * 384:(hf + 1) * 384])
                    nc.vector.tensor_add(x1_t, x1_t, xn_rb)
                    nc.sync.dma_start(
                        out=x1_sp[ttg * 128:(ttg + 1) * 128, :], in_=x1_t)
        pv.release()
        pk.release()
        pq.release()

        # ===== Phase 4: cross-attention ===================================
        pcq = tc.alloc_tile_pool(name="pcq", bufs=1)
        pck = tc.alloc_tile_pool(name="pck", bufs=1)
        pcv = tc.alloc_tile_pool(name="pcv", bufs=1)
        cq_fm = [pcq.tile([128, TQ], F32R, name=f"cqfm{dc}")
                 for dc in range(DC)]
        ck_fm = [pck.tile([128, S], F32R, name=f"ckfm{dc}")
                 for dc in range(DC)]
        cv_tok = [pcv.tile([128, H, HD + 1], BF16, name=f"cvtok{st}")
                  for st in range(S // 128)]

        with tc.tile_pool(name="px1f", bufs=1) as px1f, \
             tc.tile_pool(name="wcq", bufs=1) as wcq, \
             tc.tile_pool(name="p4a", bufs=4) as p4a, \
             tc.tile_pool(name="p4aps", bufs=6, space="PSUM") as p4aps, \
             tc.tile_pool(name="p4mps", bufs=2, space="PSUM") as p4mps:
            x1f = [px1f.tile([128, TQ], F32R, name=f"x1f{dc}")
                   for dc in range(DC)]
            for tt in range(NQT):
                x1_rb = p4a.tile([128, D], F32, name="x1rb")
                nc.sync.dma_start(out=x1_rb,
                                  in_=x1_sp[tt * 128:(tt + 1) * 128, :])
                for dc in range(DC):
                    pt = p4aps.tile([128, 128], F32, name="trp4")
                    nc.tensor.transpose(
                        pt, x1_rb[:, dc * 128:(dc + 1) * 128], ident)
                    nc.scalar.copy(
                        x1f[dc][:, tt * 128:(tt + 1) * 128], pt)
            cwq_sb = [wcq.tile([128, D], F32R, name=f"cwq{dk}")
                      for dk in range(DC)]
            for dk in range(DC):
                nc.sync.dma_start(out=cwq_sb[dk],
                                  in_=cwq[dk * 128:(dk + 1) * 128, :])
            for dc in range(DC):
                for tb in range(TQ // 512):
                    pp = p4mps.tile([128, 512], F32, name="cqpp")
                    for dk in range(DC):
                        nc.tensor.matmul(
                            pp, cwq_sb[dk][:, dc * 128:(dc + 1) * 128],
                            x1f[dk][:, tb * 512:(tb + 1) * 512],
                            start=(dk == 0), stop=(dk == DC - 1))
                    nc.scalar.activation(
                        cq_fm[dc][:, tb * 512:(tb + 1) * 512], pp,
                        AF.Identity, bias=cbq6[:, dc:dc + 1])

        with tc.tile_pool(name="penc", bufs=1) as penc, \
             tc.tile_pool(name="wck", bufs=1) as wck, \
             tc.tile_pool(name="p4kps", bufs=2, space="PSUM") as p4kps:
            enc_sb = [penc.tile([128, S], F32R, name=f"enc{dk}")
                      for dk in range(EC)]
            for dk in range(EC):
                nc.sync.dma_start(out=enc_sb[dk],
                                  in_=enc_t[dk * 128:(dk + 1) * 128, :])
            cwk_sb = [wck.tile([128, D], F32R, name=f"cwk{dk}")
                      for dk in range(EC)]
            for dk in range(EC):
                nc.sync.dma_start(out=cwk_sb[dk],
                                  in_=cwk[dk * 128:(dk + 1) * 128, :])
            for dc in range(DC):
                pp = p4kps.tile([128, 512], F32, name="ckpp")
                for dk in range(EC):
                    nc.tensor.matmul(
                        pp, cwk_sb[dk][:, dc * 128:(dc + 1) * 128],
                        enc_sb[dk], start=(dk == 0), stop=(dk == EC - 1))
                nc.scalar.activation(ck_fm[dc], pp, AF.Identity,
                                     bias=cbk6[:, dc:dc + 1])
            cwv_sb = [wck.tile([128, D], F32R, name=f"cwv{dk}")
                      for dk in range(EC)]
            for dk in range(EC):
                nc.sync.dma_start(out=cwv_sb[dk],
                                  in_=cwv[dk * 128:(dk + 1) * 128, :])
            cbv_bc = bias_bc(cbv, "cbv_bc", wck)
            for st in range(S // 128):
                vt = cv_tok[st]
                for hf in range(2):
                    pp = p4kps.tile([128, 384], F32, name="cvpp")
                    for dk in range(EC):
                        nc.tensor.matmul(
                            pp, enc_sb[dk][:, st * 128:(st + 1) * 128],
                            cwv_sb[dk][:, hf * 384:(hf + 1) * 384],
                            start=(dk == 0), stop=(dk == EC - 1))
                    nc.vector.tensor_add(
                        vt[:, hf * 6:(hf + 1) * 6, 0:HD], pp,
                        cbv_bc[:, hf * 384:(hf + 1) * 384])
                nc.vector.tensor_copy(vt[:, :, HD:HD + 1], ones12)

        with tc.tile_pool(name="wco", bufs=1) as wco, \
             tc.tile_pool(name="yp4", bufs=4) as yp4, \
             tc.tile_pool(name="pp4", bufs=4) as pp4, \
             tc.tile_pool(name="r4", bufs=2) as r4, \
             tc.tile_pool(name="ysb4", bufs=2) as ysb4, \
             tc.tile_pool(name="x14", bufs=2) as x14, \
             tc.tile_pool(name="sps4", bufs=2, space="PSUM") as sps4, \
             tc.tile_pool(name="yps4", bufs=2, space="PSUM") as yps4, \
             tc.tile_pool(name="rb4", bufs=1, space="PSUM") as rb4, \
             tc.tile_pool(name="ops4", bufs=1, space="PSUM") as ops4:
            cwo_sb = [wco.tile([128, D], F32R, name=f"cwo{dk}")
                      for dk in range(DC)]
            for dk in range(DC):
                nc.sync.dma_start(out=cwo_sb[dk],
                                  in_=cwo[dk * 128:(dk + 1) * 128, :])
            cbo_bc = bias_bc(cbo, "cbo_bc", wco)
            for qb in range(4):
                yts = [yp4.tile([128, 256], F32R, name=f"Yc{dk}")
                       for dk in range(DC)]
                for h in range(H):
                    kb, ko = h // 2, (h % 2) * 64
                    y_ps = yps4.tile([128, 256], F32, name="ypsc")
                    sps_t = sps4.tile([128, 1024], F32, name="spsc")
                    for c in range(4):
                        nc.tensor.matmul(
                            sps_t[:, c * 256:(c + 1) * 256],
                            ck_fm[kb][ko:ko + 64, c * 128:(c + 1) * 128],
                            cq_fm[kb][ko:ko + 64, qb * 256:(qb + 1) * 256],
                            start=True, stop=True)
                    p_t = pp4.tile([128, 1024], BF16, name="Pc")
                    nc.scalar.activation(p_t, sps_t, AF.Exp)
                    for c in range(4):
                        nc.tensor.matmul(y_ps[0:HD + 1, :],
                                         cv_tok[c][:, h, :],
                                         p_t[:, c * 256:(c + 1) * 256],
                                         start=(c == 0), stop=(c == 3))
                    rd = r4.tile([1, 256], F32R, name="rrc")
                    with nc.allow_low_precision(reason="softmax denom"):
                        nc.vector.reciprocal(rd[0:1, :], y_ps[64:65, :])
                    rb_ps = rb4.tile([128, 256], F32, name="rbc")
                    nc.tensor.matmul(rb_ps, ones_r1[0:1, :], rd[0:1, :],
                                     start=True, stop=True)
                    y_sb = ysb4.tile([64, 256], F32, name="ysbc")
                    nc.vector.tensor_copy(y_sb, y_ps[0:64, :])
                    nc.vector.tensor_mul(yts[kb][ko:ko + 64, :], y_sb,
                                         rb_ps[0:64, :])
                for tch in range(2):
                    ttg = qb * 2 + tch
                    x1_rb = x14.tile([128, D], F32, name="x1rb2")
                    nc.sync.dma_start(
                        out=x1_rb,
                        in_=x1_sp[ttg * 128:(ttg + 1) * 128, :])
                    x2_t = x14.tile([128, D], F32, name="x2t")
                    for hf in range(2):
                        op_ps = ops4.tile([128, 384], F32, name="opsc")
                        for dk in range(DC):
                            nc.tensor.matmul(
                                op_ps,
                                yts[dk][:, tch * 128:(tch + 1) * 128],
                                cwo_sb[dk][:, hf * 384:(hf + 1) * 384],
                                start=(dk == 0), stop=(dk == DC - 1))
                        nc.vector.tensor_add(
                            x2_t[:, hf * 384:(hf + 1) * 384], op_ps,
                            cbo_bc[:, hf * 384:(hf + 1) * 384])
                    nc.vector.tensor_add(x2_t, x2_t, x1_rb)
                    nc.sync.dma_start(
                        out=x2_sp[ttg * 128:(ttg + 1) * 128, :], in_=x2_t)
        pcv.release()
        pck.release()
        pcq.release()

        # ===== Phase 5: LN2 + MLP + residual ==============================
        ph1 = tc.alloc_tile_pool(name="ph1", bufs=1)
        h1 = [ph1.tile([128, TQ], BF16, name=f"h1_{hc}") for hc in range(HCN)]
        pw2 = tc.alloc_tile_pool(name="pw2", bufs=1)
        mw2_sb = [pw2.tile([128, D], BF16, name=f"mw2_{hc}")
                  for hc in range(HCN)]
        for hc in range(HCN):
            nc.sync.dma_start(out=mw2_sb[hc],
                              in_=mw2[hc * 128:(hc + 1) * 128, :])
        mb2_bc = bias_bc(mb2, "mb2_bc", pw2)
        with tc.tile_pool(name="pw1", bufs=1) as pw1, \
             tc.tile_pool(name="ph0", bufs=1) as ph0, \
             tc.tile_pool(name="p5a", bufs=3) as p5a, \
             tc.tile_pool(name="p5s", bufs=4) as p5s, \
             tc.tile_pool(name="p5aps", bufs=4, space="PSUM") as p5aps, \
             tc.tile_pool(name="p5mps", bufs=3, space="PSUM") as p5mps:
            mb1c = pw1.tile([128, HCN], F32, name="mb1c")
            nc.sync.dma_start(out=mb1c,
                              in_=mb1.ap().rearrange("(c p) -> p c", p=128))
            h0_fm = [ph0.tile([128, TQ], BF16, name=f"h0fm{dc}")
                     for dc in range(DC)]
            for tt in range(NQT):
                x2_rb = p5a.tile([128, D], F32, name="x2rb")
                nc.sync.dma_start(out=x2_rb,
                                  in_=x2_sp[tt * 128:(tt + 1) * 128, :])
                h0_t = p5a.tile([128, D], F32, name="h0t")
                ln_tile(p5s, x2_rb, h0_t)
                for dc in range(DC):
                    pt = p5aps.tile([128, 128], F32, name="trp5")
                    nc.tensor.transpose(
                        pt, h0_t[:, dc * 128:(dc + 1) * 128], ident)
                    nc.scalar.copy(
                        h0_fm[dc][:, tt * 128:(tt + 1) * 128], pt)
            mw1_sb = [pw1.tile([128, HID], BF16, name=f"mw1_{dk}")
                      for dk in range(DC)]
            for dk in range(DC):
                nc.sync.dma_start(out=mw1_sb[dk],
                                  in_=mw1[dk * 128:(dk + 1) * 128, :])
            for hc in range(HCN):
                for tb in range(TQ // 512):
                    pp = p5mps.tile([128, 512], F32, name="h1pp")
                    for dk in range(DC):
                        nc.tensor.matmul(
                            pp, mw1_sb[dk][:, hc * 128:(hc + 1) * 128],
                            h0_fm[dk][:, tb * 512:(tb + 1) * 512],
                            start=(dk == 0), stop=(dk == DC - 1))
                    nc.scalar.activation(
                        h1[hc][:, tb * 512:(tb + 1) * 512], pp, AF.Gelu,
                        bias=mb1c[:, hc:hc + 1])
        with tc.tile_pool(name="p5o", bufs=3) as p5o, \
             tc.tile_pool(name="p5ops", bufs=3, space="PSUM") as p5ops:
            for tt in range(NQT):
                x2_rb = p5o.tile([128, D], F32, name="x2rb2")
                nc.sync.dma_start(out=x2_rb,
                                  in_=x2_sp[tt * 128:(tt + 1) * 128, :])
                o_t = p5o.tile([128, D], F32, name="o_t")
                for hf in range(2):
                    pp = p5ops.tile([128, 384], F32, name="opp")
                    for hc in range(HCN):
                        nc.tensor.matmul(
                            pp, h1[hc][:, tt * 128:(tt + 1) * 128],
                            mw2_sb[hc][:, hf * 384:(hf + 1) * 384],
                            start=(hc == 0), stop=(hc == HCN - 1))
                    nc.vector.tensor_add(
                        o_t[:, hf * 384:(hf + 1) * 384], pp,
                        mb2_bc[:, hf * 384:(hf + 1) * 384])
                nc.vector.tensor_add(o_t, o_t, x2_rb)
                nc.sync.dma_start(out=out_own[tt * 128:(tt + 1) * 128, :],
                                  in_=o_t)
        pw2.release()
        ph1.release()
        dram.release()
        singles.release()

    nc.compile()
    return nc


def _get_nc():
    if "nc" not in _CACHE:
        _CACHE["nc"] = _build()
    return _CACHE["nc"]


def _make_in_maps(inputs):
    x = np.asarray(inputs["x"], np.float32)
    enc = np.asarray(inputs["encoder_hidden_states"], np.float32)
    scale = np.float32(1.0 / np.sqrt(HD))

    f32 = lambda a: np.ascontiguousarray(np.asarray(a, np.float32))
    bf = lambda a: np.ascontiguousarray(
        np.asarray(a, np.float32).astype(ml_dtypes.bfloat16))

    g1 = np.asarray(inputs["ln1_g"], np.float64)
    b1 = np.asarray(inputs["ln1_b"], np.float64)
    g2 = np.asarray(inputs["ln2_g"], np.float64)
    b2 = np.asarray(inputs["ln2_b"], np.float64)
    sWq = np.asarray(inputs["sWq"], np.float64)
    sWk = np.asarray(inputs["sWk"], np.float64)
    sWv = np.asarray(inputs["sWv"], np.float64)
    mW1 = np.asarray(inputs["mW1"], np.float64)
    shared = dict(
        wq=f32(g1[:, None] * sWq * scale),
        bq=f32((b1 @ sWq + np.asarray(inputs["sbq"], np.float64)) * scale),
        wk=f32(g1[:, None] * sWk),
        bk=f32(b1 @ sWk + np.asarray(inputs["sbk"], np.float64)),
        wv=f32(g1[:, None] * sWv),
        bv=f32(b1 @ sWv + np.asarray(inputs["sbv"], np.float64)),
        wo=f32(inputs["sWo"]), bo=f32(inputs["sbo"]),
        cwq=f32(inputs["cWq"]) * scale, cbq=f32(inputs["cbq"]) * scale,
        cwk=f32(inputs["cWk"]), cbk=f32(inputs["cbk"]),
        cwv=f32(inputs["cWv"]), cbv=f32(inputs["cbv"]),
        cwo=f32(inputs["cWo"]), cbo=f32(inputs["cbo"]),
        mw1=bf(g2[:, None] * mW1),
        mb1=f32(np.asarray(inputs["mb1"], np.float64) + b2 @ mW1),
        mw2=bf(inputs["mW2"]), mb2=f32(inputs["mb2"]),
        ln1g=f32(inputs["ln1_g"]), ln1b=f32(inputs["ln1_b"]),
        kg_cols=np.ascontiguousarray(
            np.arange(T, dtype=np.float32).reshape(NTT, 128).T),
    )
    in_maps = []
    for c in range(8):
        b, p = c // 2, c % 2
        m = dict(shared)
        m["x_full"] = np.ascontiguousarray(x[b])
        m["x_own"] = np.ascontiguousarray(x[b, p::2])
        m["enc_t"] = np.ascontiguousarray(enc[b].T)
        m["qg_row"] = np.ascontiguousarray(
            (2.0 * np.arange(TQ, dtype=np.float32) + p).reshape(1, TQ))
        in_maps.append(m)
    return in_maps


def kernel(**inputs):
    in_maps = _make_in_maps(inputs)
    nc = _get_nc()
    res = run_bass_kernel_spmd(nc, in_maps, core_ids=list(range(8)))
    out = np.empty((B, T, NINP), np.float32)
    for c in range(8):
        b, p = c // 2, c % 2
        out[b, p::2] = res.results[c]["out_own"]
    return out



# revision 13
# speedup vs baseline: 1.6502x; 1.6502x over previous
"""Trainium2 Bass kernel for a dense transformer block (self-attn causal +
cross-attn + MLP), sharded over 8 NeuronCores without collectives.

Sharding: core c -> batch b = c//2, parity p = c%2. Each core computes the
output for query rows p::2 of batch b (1024 rows). K/V for self-attention are
recomputed per-core for the full 2048-row sequence. Parity is handled host-
side: for p=1 the sequence rows are pair-swapped so own tokens sit at even
positions for every core (one program for all 8 cores); the causal mask input
compensates for the within-pair key reordering.

v2: fp8e4m3 DoubleRow matmuls for all projections/PV/MLP, bf16 scores,
feature-major bf16 residual chain (SBUF-resident, no DRAM spills), causal mask
via PE identity-add of an additive mask into score PSUM, exp straight to fp8
with a -2 bias, softmax denominator reciprocal broadcast via SBUF->SBUF DMA,
LN2 computed feature-major with ones-matmul partition reductions. Power-of-2
scales keep fp8 operands out of the subnormal range; the scales unfold for
free inside the psum->sbuf bias copies.
"""
import sys

sys.path.insert(0, "/opt/trn_rl_repo")

import numpy as np
import ml_dtypes

import concourse.bass as bass
import concourse.tile as tile
from concourse import bacc, mybir
from concourse.bass_utils import run_bass_kernel_spmd
from concourse.masks import make_identity

F32 = mybir.dt.float32
BF16 = mybir.dt.bfloat16
FP8 = mybir.dt.float8e4
AF = mybir.ActivationFunctionType
OP = mybir.AluOpType
DR = mybir.MatmulPerfMode.DoubleRow

B, T, S, D = 4, 2048, 512, 768
NINP = 768
H, HD, HID = 12, 64, 3072
TQ = T // 2            # own query rows per core
DC = D // 128          # 6 feature chunks
HCN = HID // 128       # 24 hidden chunks
EPS = 1e-5

SW = 4096.0            # fp8 scale for most weights
SW2 = 8192.0           # fp8 scale for mw2
SE = 32.0              # fp8 scale for encoder activations
SX = 16.0              # fp8 scale for xn / x1 / h0 / y activations
ISXW = 1.0 / (SX * SW)
ISEW = 1.0 / (SE * SW)

_CACHE: dict = {}

f8 = ml_dtypes.float8_e4m3


def _bc(ap, n):
    """Partition-broadcast AP of a [1, n] slice."""
    return bass.AP(tensor=ap.tensor, offset=ap.offset, ap=[[0, 128], [1, n]])


def _build():
    nc = bacc.Bacc("TRN2", target_bir_lowering=False, debug=False)

    x_full = nc.dram_tensor("x_full", [T, D], BF16, kind="ExternalInput")
    wqp = nc.dram_tensor("wqp", [3 * 128, 2 * D], FP8, kind="ExternalInput")
    wkp = nc.dram_tensor("wkp", [3 * 128, 2 * D], FP8, kind="ExternalInput")
    wvp = nc.dram_tensor("wvp", [3 * 128, 2 * D], FP8, kind="ExternalInput")
    wop = nc.dram_tensor("wop", [3 * 128, 2 * D], FP8, kind="ExternalInput")
    cwqp = nc.dram_tensor("cwqp", [3 * 128, 2 * D], FP8, kind="ExternalInput")
    cwkp = nc.dram_tensor("cwkp", [4 * 128, 2 * D], FP8, kind="ExternalInput")
    cwvp = nc.dram_tensor("cwvp", [4 * 128, 2 * D], FP8, kind="ExternalInput")
    cwop = nc.dram_tensor("cwop", [3 * 128, 2 * D], FP8, kind="ExternalInput")
    mw1p = nc.dram_tensor("mw1p", [3 * 128, 2 * HID], FP8, kind="ExternalInput")
    mw2p = nc.dram_tensor("mw2p", [12 * 128, 2 * D], FP8, kind="ExternalInput")
    encp = nc.dram_tensor("encp", [4 * 128, 2 * S], FP8, kind="ExternalInput")
    bq = nc.dram_tensor("bq", [D], F32, kind="ExternalInput")
    bk = nc.dram_tensor("bk", [D], F32, kind="ExternalInput")
    bv = nc.dram_tensor("bv", [D], F32, kind="ExternalInput")
    cbq = nc.dram_tensor("cbq", [D], F32, kind="ExternalInput")
    cbk = nc.dram_tensor("cbk", [D], F32, kind="ExternalInput")
    cbv = nc.dram_tensor("cbv", [D], F32, kind="ExternalInput")
    mb1 = nc.dram_tensor("mb1", [HID], F32, kind="ExternalInput")
    mb2x = nc.dram_tensor("mb2x", [D], F32, kind="ExternalInput")  # mb2*SW2
    resg = nc.dram_tensor("resg", [D], F32, kind="ExternalInput")  # g1
    resb = nc.dram_tensor("resb", [D], F32, kind="ExternalInput")  # b1+bo+cbo
    maskq = nc.dram_tensor("maskq", [128, 1024], BF16, kind="ExternalInput")
    out_own = nc.dram_tensor("out_own", [TQ, D], BF16, kind="ExternalOutput")

    with tile.TileContext(nc) as tc:
        # pool stack; release order is the reverse of allocation order
        singles = tc.alloc_tile_pool(name="singles", bufs=1)
        pX2 = tc.alloc_tile_pool(name="pX2", bufs=1)       # to end
        pC = tc.alloc_tile_pool(name="pC", bufs=1)         # to end of ph4
        pX1 = tc.alloc_tile_pool(name="pX1", bufs=1)       # to end of ph4
        pQKV = tc.alloc_tile_pool(name="pQKV", bufs=1)     # to end of ph3
        pXN = tc.alloc_tile_pool(name="pXN", bufs=1)       # to end of ph3

        identf = singles.tile([128, 128], F32, name="identf")
        make_identity(nc, identf[:, :])
        identb = singles.tile([128, 128], BF16, name="identb")
        nc.vector.tensor_copy(identb, identf)
        eps_t = singles.tile([128, 1], F32, name="eps")
        nc.vector.memset(eps_t, EPS)
        neg2 = singles.tile([128, 1], F32, name="neg2")
        nc.vector.memset(neg2, -2.0)
        eps256 = singles.tile([1, 1], F32, name="eps256")
        nc.vector.memset(eps256, EPS / 256.0)
        ones1b = singles.tile([128, 1], BF16, name="ones1b")
        nc.vector.memset(ones1b, 1.0)
        onesrow = singles.tile([1, 128], BF16, name="onesrow")
        nc.vector.memset(onesrow, 1.0)
        mask_sb = singles.tile([128, 1024], BF16, name="mask_sb")
        nc.sync.dma_start(out=mask_sb, in_=maskq[:, :])

        def bias6(h, name, pool=None):
            n = h.shape[0]
            t = (pool or singles).tile([128, n // 128], F32, name=name)
            nc.sync.dma_start(out=t, in_=h.ap().rearrange("(c p) -> p c", p=128))
            return t

        def bias_bc(h, name, pool, n=D):
            t = pool.tile([128, n], F32, name=name)
            nc.gpsimd.dma_start(out=t, in_=_bc(h.ap(), n))
            return t

        bq6 = bias6(bq, "bq6")
        bk6 = bias6(bk, "bk6")
        cbq6 = bias6(cbq, "cbq6")
        cbk6 = bias6(cbk, "cbk6")
        g6 = bias6(resg, "g6")
        rb6 = bias6(resb, "rb6")
        mb2x6 = bias6(mb2x, "mb2x6")

        # persistent activation tiles
        x2Tb = [pX2.tile([128, TQ], BF16, name=f"x2Tb{dc}") for dc in range(DC)]
        cqT = [pC.tile([128, TQ], BF16, name=f"cqT{dc}") for dc in range(DC)]
        ckT = [pC.tile([128, S], BF16, name=f"ckT{dc}") for dc in range(DC)]
        cvP = [pC.tile([128, 2, H, HD + 1], FP8, name=f"cvP{i}") for i in range(2)]
        x1Tb = [pX1.tile([128, TQ], BF16, name=f"x1Tb{dc}") for dc in range(DC)]
        x1T8 = [pX1.tile([128, 2, TQ], FP8, name=f"x1T8{j}") for j in range(3)]
        qT = [pQKV.tile([128, TQ], BF16, name=f"qT{dc}") for dc in range(DC)]
        kT = [pQKV.tile([128, T], BF16, name=f"kT{dc}") for dc in range(DC)]
        vP = [pQKV.tile([128, 2, H, HD + 1], FP8, name=f"vP{i}") for i in range(8)]
        xnT8 = [pXN.tile([128, 2, T], FP8, name=f"xnT8{j}") for j in range(3)]
        xnTb = [pXN.tile([128, TQ], BF16, name=f"xnTb{dc}") for dc in range(DC)]

        # ===== Phase 1: LN1 -> transposes -> Q/K/V projections ============
        with tc.tile_pool(name="w1", bufs=1) as w1, \
             tc.tile_pool(name="p1", bufs=4) as p1, \
             tc.tile_pool(name="p1s", bufs=6) as p1s, \
             tc.tile_pool(name="p1tp", bufs=1, space="PSUM") as p1tp, \
             tc.tile_pool(name="p1mm", bufs=2, space="PSUM") as p1mm:
            wq_sb = [w1.tile([128, 2, D], FP8, name=f"wq{j}") for j in range(3)]
            wk_sb = [w1.tile([128, 2, D], FP8, name=f"wk{j}") for j in range(3)]
            wv_sb = [w1.tile([128, 2, D], FP8, name=f"wv{j}") for j in range(3)]
            for j in range(3):
                nc.sync.dma_start(out=wq_sb[j], in_=wqp[j * 128:(j + 1) * 128, :])
                nc.sync.dma_start(out=wk_sb[j], in_=wkp[j * 128:(j + 1) * 128, :])
                nc.sync.dma_start(out=wv_sb[j], in_=wvp[j * 128:(j + 1) * 128, :])
            bv_bc = bias_bc(bv, "bv_bc", w1)
            for cp in range(8):
                nc.gpsimd.memset(vP[cp][:, :, :, HD:HD + 1], 1.0 / SX)

            for blk in range(4):  # 512-token blocks of the full sequence
                psT = [p1tp.tile([128, 2, 512], BF16, name=f"psT{j}")
                       for j in range(3)]
                for t4 in range(4):
                    tt = blk * 4 + t4
                    xt = p1.tile([128, D], BF16, name="xt")
                    nc.sync.dma_start(
                        out=xt, in_=x_full[tt * 128:(tt + 1) * 128, :])
                    xr = xt.rearrange("p (s f) -> p s f", f=256)
                    stats = p1s.tile([128, 3, 6], F32, name="bnst")
                    for si in range(3):
                        nc.vector.bn_stats(out=stats[:, si, :], in_=xr[:, si, :])
                    mv = p1s.tile([128, 2], F32, name="bnmv")
                    nc.vector.bn_aggr(out=mv, in_=stats)
                    std = p1s.tile([128, 1], F32, name="std")
                    nc.scalar.activation(std, mv[:, 1:2], AF.Sqrt, bias=eps_t)
                    rstd = p1s.tile([128, 1], F32, name="rstd")
                    nc.vector.reciprocal(rstd, std)
                    xnt = p1.tile([128, D], BF16, name="xnt")
                    nc.vector.tensor_scalar(xnt, xt, mv[:, 0:1], rstd,
                                            OP.subtract, OP.mult)
                    for dc in range(DC):
                        nc.tensor.transpose(
                            psT[dc // 2][:, dc % 2, t4 * 128:(t4 + 1) * 128],
                            xnt[:, dc * 128:(dc + 1) * 128], identb)
                # psum -> sbuf: fp8 (x SX) for matmuls; bf16 affine residual
                # (own = even columns after the host parity permutation)
                for j in range(3):
                    dst8 = xnT8[j][:, :, blk * 512:(blk + 1) * 512]
                    if j == 0:
                        nc.scalar.mul(dst8, psT[j], SX)
                    elif j == 1:
                        nc.gpsimd.tensor_scalar(dst8, psT[j], SX, None, OP.mult)
                    else:
                        nc.vector.tensor_scalar(dst8, psT[j], SX, None, OP.mult)
                for dc in range(DC):
                    nc.gpsimd.tensor_scalar(
                        xnTb[dc][:, blk * 256:(blk + 1) * 256],
                        psT[dc // 2][:, dc % 2, 0:512:2],
                        g6[:, dc:dc + 1], rb6[:, dc:dc + 1],
                        OP.mult, OP.add)
                # K projection for this block
                for dc in range(DC):
                    pp = p1mm.tile([128, 512], F32, name="kpp")
                    for half in range(2):
                        for j in range(3):
                            nc.tensor.matmul(
                                pp[:, half * 256:(half + 1) * 256],
                                wk_sb[j][:, :, dc * 128:(dc + 1) * 128],
                                xnT8[j][:, :, blk * 512 + half * 256:
                                        blk * 512 + (half + 1) * 256],
                                start=(j == 0), stop=(j == 2), perf_mode=DR)
                    nc.vector.tensor_scalar(
                        kT[dc][:, blk * 512:(blk + 1) * 512], pp,
                        ISXW, bk6[:, dc:dc + 1], OP.mult, OP.add)
                # V projection for this block
                for t4 in range(4):
                    tt = blk * 4 + t4
                    for hf in range(2):
                        pp = p1mm.tile([128, 384], F32, name="vpp")
                        for j in range(3):
                            nc.tensor.matmul(
                                pp,
                                xnT8[j][:, :, tt * 128:(tt + 1) * 128],
                                wv_sb[j][:, :, hf * 384:(hf + 1) * 384],
                                start=(j == 0), stop=(j == 2), perf_mode=DR)
                        nc.gpsimd.scalar_tensor_tensor(
                            vP[tt // 2][:, tt % 2, hf * 6:(hf + 1) * 6, 0:HD],
                            pp, ISXW, bv_bc[:, hf * 384:(hf + 1) * 384],
                            OP.mult, OP.add)
            # Q projection (own = even columns, strided)
            for dc in range(DC):
                for qblk in range(2):
                    pp = p1mm.tile([128, 512], F32, name="kpp")
                    for half in range(2):
                        base = qblk * 1024 + half * 512
                        for j in range(3):
                            nc.tensor.matmul(
                                pp[:, half * 256:(half + 1) * 256],
                                wq_sb[j][:, :, dc * 128:(dc + 1) * 128],
                                xnT8[j][:, :, base:base + 512:2],
                                start=(j == 0), stop=(j == 2), perf_mode=DR)
                    nc.vector.tensor_scalar(
                        qT[dc][:, qblk * 512:(qblk + 1) * 512], pp,
                        ISXW, bq6[:, dc:dc + 1], OP.mult, OP.add)

        # ===== Phase 3: causal self-attention =============================
        with tc.tile_pool(name="w3", bufs=1) as w3, \
             tc.tile_pool(name="y8p", bufs=2) as y8p, \
             tc.tile_pool(name="ytm3", bufs=2) as ytm3, \
             tc.tile_pool(name="pp3", bufs=4) as pp3, \
             tc.tile_pool(name="sps3", bufs=2, space="PSUM") as sps3, \
             tc.tile_pool(name="yps3", bufs=1, space="PSUM") as yps3, \
             tc.tile_pool(name="ptp3", bufs=1, space="PSUM") as ptp3, \
             tc.tile_pool(name="ops3", bufs=2, space="PSUM") as ops3:
            wo_sb = [w3.tile([128, 2, D], FP8, name=f"wo{j}") for j in range(3)]
            for j in range(3):
                nc.sync.dma_start(out=wo_sb[j], in_=wop[j * 128:(j + 1) * 128, :])
            for qb in range(4):
                ng = qb + 1
                yT8 = [y8p.tile([128, 2, 256], FP8, name=f"yT8{j}")
                       for j in range(3)]
                ytm = [ytm3.tile([128, D], BF16, name=f"ytm{qh}")
                       for qh in range(2)]
                for h in range(H):
                    kb, ko = h // 2, (h % 2) * 64
                    y_ps = yps3.tile([128, 2, HD + 1], F32, name="yps")
                    for g in range(ng):
                        sps = sps3.tile([128, 4, 256], F32, name="sps")
                        for c in range(4):
                            nc.tensor.matmul(
                                sps[:, c, :],
                                kT[kb][ko:ko + 64,
                                       (g * 4 + c) * 128:(g * 4 + c + 1) * 128],
                                qT[kb][ko:ko + 64, qb * 256:(qb + 1) * 256],
                                start=True, stop=(g < ng - 1))
                        if g == ng - 1:  # diagonal group: additive causal mask
                            for half in range(2):
                                nc.tensor.matmul(
                                    sps[:, half * 2:(half + 1) * 2, :],
                                    identb,
                                    mask_sb[:, half * 512:(half + 1) * 512],
                                    start=False, stop=True,
                                    skip_group_check=True)
                        p_t = pp3.tile([128, 4, 256], FP8, name="P")
                        nc.scalar.activation(p_t, sps, AF.Exp, bias=neg2)
                        for qh in range(2):
                            for j2 in range(2):
                                nc.tensor.matmul(
                                    y_ps[:, qh, :],
                                    p_t[:, j2 * 2:(j2 + 1) * 2,
                                        qh * 128:(qh + 1) * 128],
                                    vP[g * 2 + j2][:, :, h, :],
                                    start=(g == 0 and j2 == 0),
                                    stop=(g == ng - 1 and j2 == 1),
                                    perf_mode=DR)
                    for qh in range(2):
                        with nc.allow_low_precision(reason="softmax denom"):
                            nc.gpsimd.tensor_scalar(
                                ytm[qh][:, h * HD:(h + 1) * HD],
                                y_ps[:, qh, 0:HD], y_ps[:, qh, HD:HD + 1],
                                None, OP.divide)
                # transpose y back to feature-major fp8 pairs
                for qh in range(2):
                    ptT = ptp3.tile([128, D], BF16, name="ptT")
                    for dc in range(DC):
                        nc.tensor.transpose(
                            ptT[:, dc * 128:(dc + 1) * 128],
                            ytm[qh][:, dc * 128:(dc + 1) * 128], identb)
                    for j in range(3):
                        nc.vector.tensor_scalar(
                            yT8[j][:, :, qh * 128:(qh + 1) * 128],
                            ptT[:, j * 256:(j + 1) * 256], SX, None, OP.mult)
                # O projection + residual (feature-major)
                for oc in range(DC):
                    xo = ops3.tile([128, 256], F32, name="xo")
                    for j in range(3):
                        nc.tensor.matmul(
                            xo, wo_sb[j][:, :, oc * 128:(oc + 1) * 128],
                            yT8[j], start=(j == 0), stop=(j == 2),
                            perf_mode=DR)
                    nc.vector.scalar_tensor_tensor(
                        x1Tb[oc][:, qb * 256:(qb + 1) * 256],
                        xo, ISXW, xnTb[oc][:, qb * 256:(qb + 1) * 256],
                        OP.mult, OP.add)
                    nc.gpsimd.tensor_scalar(
                        x1T8[oc // 2][:, oc % 2, qb * 256:(qb + 1) * 256],
                        x1Tb[oc][:, qb * 256:(qb + 1) * 256],
                        SX, None, OP.mult)
        pXN.release()
        pQKV.release()

        # ===== Phase 4: cross-attention ===================================
        with tc.tile_pool(name="w4", bufs=1) as w4, \
             tc.tile_pool(name="y4p", bufs=2) as y4p, \
             tc.tile_pool(name="ytm4", bufs=2) as ytm4, \
             tc.tile_pool(name="pp4", bufs=4) as pp4:
            cwq_sb = [w4.tile([128, 2, D], FP8, name=f"cwq{j}") for j in range(3)]
            cwk_sb = [w4.tile([128, 2, D], FP8, name=f"cwk{j}") for j in range(4)]
            cwv_sb = [w4.tile([128, 2, D], FP8, name=f"cwv{j}") for j in range(4)]
            cwo_sb = [w4.tile([128, 2, D], FP8, name=f"cwo{j}") for j in range(3)]
            enc_sb = [w4.tile([128, 2, S], FP8, name=f"enc{j}") for j in range(4)]
            for j in range(3):
                nc.sync.dma_start(out=cwq_sb[j], in_=cwqp[j * 128:(j + 1) * 128, :])
                nc.sync.dma_start(out=cwo_sb[j], in_=cwop[j * 128:(j + 1) * 128, :])
            for j in range(4):
                nc.sync.dma_start(out=cwk_sb[j], in_=cwkp[j * 128:(j + 1) * 128, :])
                nc.sync.dma_start(out=cwv_sb[j], in_=cwvp[j * 128:(j + 1) * 128, :])
                nc.sync.dma_start(out=enc_sb[j], in_=encp[j * 128:(j + 1) * 128, :])
            cbv_bc = bias_bc(cbv, "cbv_bc", w4)
            for i in range(2):
                nc.gpsimd.memset(cvP[i][:, :, :, HD:HD + 1], 1.0 / SX)
            with tc.tile_pool(name="prj4", bufs=2, space="PSUM") as prj4, \
                 tc.tile_pool(name="prv4", bufs=2, space="PSUM") as prv4:
                for dc in range(DC):
                    for qblk in range(2):
                        pp = prj4.tile([128, 512], F32, name="prjp")
                        for half in range(2):
                            base = qblk * 512 + half * 256
                            for j in range(3):
                                nc.tensor.matmul(
                                    pp[:, half * 256:(half + 1) * 256],
                                    cwq_sb[j][:, :, dc * 128:(dc + 1) * 128],
                                    x1T8[j][:, :, base:base + 256],
                                    start=(j == 0), stop=(j == 2), perf_mode=DR)
                        nc.vector.tensor_scalar(
                            cqT[dc][:, qblk * 512:(qblk + 1) * 512], pp,
                            ISXW, cbq6[:, dc:dc + 1], OP.mult, OP.add)
                for dc in range(DC):
                    pp = prj4.tile([128, 512], F32, name="prjp")
                    for half in range(2):
                        for j in range(4):
                            nc.tensor.matmul(
                                pp[:, half * 256:(half + 1) * 256],
                                cwk_sb[j][:, :, dc * 128:(dc + 1) * 128],
                                enc_sb[j][:, :, half * 256:(half + 1) * 256],
                                start=(j == 0), stop=(j == 3), perf_mode=DR)
                    nc.vector.tensor_scalar(
                        ckT[dc], pp, ISEW, cbk6[:, dc:dc + 1], OP.mult, OP.add)
                for st in range(4):
                    for hf in range(2):
                        pp = prv4.tile([128, 384], F32, name="cvpp")
                        for j in range(4):
                            nc.tensor.matmul(
                                pp, enc_sb[j][:, :, st * 128:(st + 1) * 128],
                                cwv_sb[j][:, :, hf * 384:(hf + 1) * 384],
                                start=(j == 0), stop=(j == 3), perf_mode=DR)
                        nc.gpsimd.scalar_tensor_tensor(
                            cvP[st // 2][:, st % 2, hf * 6:(hf + 1) * 6, 0:HD],
                            pp, ISEW, cbv_bc[:, hf * 384:(hf + 1) * 384],
                            OP.mult, OP.add)
            with tc.tile_pool(name="sps4", bufs=2, space="PSUM") as sps4, \
                 tc.tile_pool(name="yps4", bufs=1, space="PSUM") as yps4, \
                 tc.tile_pool(name="ptp4", bufs=1, space="PSUM") as ptp4, \
                 tc.tile_pool(name="ops4", bufs=2, space="PSUM") as ops4:
                for qb in range(4):
                    yT8 = [y4p.tile([128, 2, 256], FP8, name=f"yc8{j}")
                           for j in range(3)]
                    ytm = [ytm4.tile([128, D], BF16, name=f"ycm{qh}")
                           for qh in range(2)]
                    for h in range(H):
                        kb, ko = h // 2, (h % 2) * 64
                        y_ps = yps4.tile([128, 2, HD + 1], F32, name="ypsc")
                        sps = sps4.tile([128, 4, 256], F32, name="spsc")
                        for c in range(4):
                            nc.tensor.matmul(
                                sps[:, c, :],
                                ckT[kb][ko:ko + 64, c * 128:(c + 1) * 128],
                                cqT[kb][ko:ko + 64, qb * 256:(qb + 1) * 256],
                                start=True, stop=True)
                        p_t = pp4.tile([128, 4, 256], FP8, name="Pc")
                        nc.scalar.activation(p_t, sps, AF.Exp, bias=neg2)
                        for qh in range(2):
                            for j2 in range(2):
                                nc.tensor.matmul(
                                    y_ps[:, qh, :],
                                    p_t[:, j2 * 2:(j2 + 1) * 2,
                                        qh * 128:(qh + 1) * 128],
                                    cvP[j2][:, :, h, :],
                                    start=(j2 == 0), stop=(j2 == 1),
                                    perf_mode=DR)
                        for qh in range(2):
                            with nc.allow_low_precision(reason="softmax denom"):
                                nc.gpsimd.tensor_scalar(
                                    ytm[qh][:, h * HD:(h + 1) * HD],
                                    y_ps[:, qh, 0:HD], y_ps[:, qh, HD:HD + 1],
                                    None, OP.divide)
                    for qh in range(2):
                        ptT = ptp4.tile([128, D], BF16, name="ptTc")
                        for dc in range(DC):
                            nc.tensor.transpose(
                                ptT[:, dc * 128:(dc + 1) * 128],
                                ytm[qh][:, dc * 128:(dc + 1) * 128], identb)
                        for j in range(3):
                            nc.vector.tensor_scalar(
                                yT8[j][:, :, qh * 128:(qh + 1) * 128],
                                ptT[:, j * 256:(j + 1) * 256], SX, None,
                                OP.mult)
                    for oc in range(DC):
                        xo = ops4.tile([128, 256], F32, name="xoc")
                        for j in range(3):
                            nc.tensor.matmul(
                                xo, cwo_sb[j][:, :, oc * 128:(oc + 1) * 128],
                                yT8[j], start=(j == 0), stop=(j == 2),
                                perf_mode=DR)
                        nc.vector.scalar_tensor_tensor(
                            x2Tb[oc][:, qb * 256:(qb + 1) * 256],
                            xo, ISXW, x1Tb[oc][:, qb * 256:(qb + 1) * 256],
                            OP.mult, OP.add)
        pX1.release()
        pC.release()

        # ===== Phase 5: LN2 (feature-major) + MLP + out ===================
        with tc.tile_pool(name="w5", bufs=1) as w5, \
             tc.tile_pool(name="p5a", bufs=1) as p5a, \
             tc.tile_pool(name="p5b", bufs=3) as p5b, \
             tc.tile_pool(name="h0p", bufs=1) as h0p, \
             tc.tile_pool(name="h1p", bufs=1) as h1p, \
             tc.tile_pool(name="oTp", bufs=1) as oTp:
            mw1_sb = [w5.tile([128, 2, HID], FP8, name=f"mw1_{j}")
                      for j in range(3)]
            mw2_sb = [w5.tile([128, 2, D], FP8, name=f"mw2_{j}")
                      for j in range(12)]
            for j in range(3):
                nc.sync.dma_start(out=mw1_sb[j], in_=mw1p[j * 128:(j + 1) * 128, :])
            for j in range(12):
                nc.sync.dma_start(out=mw2_sb[j], in_=mw2p[j * 128:(j + 1) * 128, :])
            mb1c = w5.tile([128, HCN], F32, name="mb1c")
            nc.sync.dma_start(out=mb1c,
                              in_=mb1.ap().rearrange("(c p) -> p c", p=128))
            h0T8 = [h0p.tile([128, 2, TQ], FP8, name=f"h0T8{j}")
                    for j in range(3)]
            # LN2 stats via ones-matmul partition reduction
            with tc.tile_pool(name="p5st", bufs=1, space="PSUM") as p5st, \
                 tc.tile_pool(name="p5bc", bufs=1, space="PSUM") as p5bc:
                s1 = p5st.tile([1, TQ], F32, name="s1")
                s2 = p5st.tile([1, TQ], F32, name="s2")
                for blk2 in range(2):
                    sl = slice(blk2 * 512, (blk2 + 1) * 512)
                    for dc in range(DC):
                        nc.tensor.matmul(s1[0:1, sl], ones1b, x2Tb[dc][:, sl],
                                         start=(dc == 0), stop=(dc == DC - 1))
                    for dc in range(DC):
                        sq = p5b.tile([128, 512], BF16, name="sq")
                        nc.vector.tensor_mul(sq, x2Tb[dc][:, sl],
                                             x2Tb[dc][:, sl])
                        nc.tensor.matmul(s2[0:1, sl], ones1b, sq,
                                         start=(dc == 0), stop=(dc == DC - 1))
                mu_n = p5a.tile([1, TQ], F32, name="mu_n")
                nc.vector.tensor_scalar(mu_n, s1, -1.0 / D, None, OP.mult)
                msq = p5a.tile([1, TQ], F32, name="msq")
                nc.vector.tensor_scalar(msq, s2, 1.0 / D, None, OP.mult)
                mu2 = p5a.tile([1, TQ], F32, name="mu2")
                nc.vector.tensor_mul(mu2, mu_n, mu_n)
                var = p5a.tile([1, TQ], F32, name="var")
                nc.vector.tensor_sub(var, msq, mu2)
                # std16 = sqrt((var+eps)/256) = std/16 ; a = 1/std16 = 16*rstd
                std16 = p5a.tile([1, TQ], F32, name="std16")
                nc.scalar.activation(std16, var, AF.Sqrt, bias=eps256,
                                     scale=1.0 / 256.0)
                a_f = p5a.tile([1, TQ], F32, name="a_f")
                nc.vector.reciprocal(a_f, std16)
                a_b = p5a.tile([1, TQ], BF16, name="a_b")
                nc.vector.tensor_copy(a_b, a_f)
                c_b = p5a.tile([1, TQ], BF16, name="c_b")
                nc.vector.tensor_mul(c_b, mu_n, a_f)
                a_bc = p5bc.tile([128, TQ], F32, name="a_bc")
                c_bc = p5bc.tile([128, TQ], F32, name="c_bc")
                for blk2 in range(2):
                    sl = slice(blk2 * 512, (blk2 + 1) * 512)
                    nc.tensor.matmul(a_bc[:, sl], onesrow, a_b[0:1, sl],
                                     start=True, stop=True)
                    nc.tensor.matmul(c_bc[:, sl], onesrow, c_b[0:1, sl],
                                     start=True, stop=True)
                for dc in range(DC):
                    tmp = p5b.tile([128, TQ], BF16, name="h0tmp")
                    nc.vector.tensor_mul(tmp, x2Tb[dc], a_bc)
                    nc.gpsimd.tensor_tensor(
                        h0T8[dc // 2][:, dc % 2, :], tmp, c_bc, OP.add)
            # h1 = gelu((mw1^T h0 + mb1)) -> fp8
            h1T8 = [h1p.tile([128, 2, TQ], FP8, name=f"h1T8{j}")
                    for j in range(12)]
            with tc.tile_pool(name="p5m1", bufs=2, space="PSUM") as p5m1:
                for hc in range(HCN):
                    pp = p5m1.tile([128, TQ], F32, name="h1pp")
                    for blk2 in range(2):
                        for half in range(2):
                            sl = slice(blk2 * 512 + half * 256,
                                       blk2 * 512 + (half + 1) * 256)
                            for j in range(3):
                                nc.tensor.matmul(
                                    pp[:, sl],
                                    mw1_sb[j][:, :, hc * 128:(hc + 1) * 128],
                                    h0T8[j][:, :, sl],
                                    start=(j == 0), stop=(j == 2),
                                    perf_mode=DR)
                    nc.scalar.activation(
                        h1T8[hc // 2][:, hc % 2, :], pp, AF.Gelu,
                        bias=mb1c[:, hc:hc + 1], scale=ISXW)
            # h2 + mb2 + residual -> outT (feature-major)
            outT = [oTp.tile([128, TQ], BF16, name=f"outT{dc}")
                    for dc in range(DC)]
            with tc.tile_pool(name="p5m2", bufs=2, space="PSUM") as p5m2:
                for oc in range(DC):
                    for blk2 in range(2):
                        sl = slice(blk2 * 512, (blk2 + 1) * 512)
                        pp = p5m2.tile([128, 512], F32, name="h2pp")
                        for half in range(2):
                            sl2 = slice(blk2 * 512 + half * 256,
                                        blk2 * 512 + (half + 1) * 256)
                            for j in range(12):
                                nc.tensor.matmul(
                                    pp[:, half * 256:(half + 1) * 256],
                                    mw2_sb[j][:, :, oc * 128:(oc + 1) * 128],
                                    h1T8[j][:, :, sl2],
                                    start=(j == 0), stop=(j == 11),
                                    perf_mode=DR)
                        t5 = p5b.tile([128, 512], F32, name="t5")
                        nc.vector.tensor_scalar(
                            t5, pp, mb2x6[:, oc:oc + 1], 1.0 / SW2,
                            OP.add, OP.mult)
                        nc.gpsimd.tensor_tensor(
                            outT[oc][:, sl], t5, x2Tb[oc][:, sl], OP.add)
            # transpose back to token-major + DMA out
            with tc.tile_pool(name="p5tp", bufs=2, space="PSUM") as p5tp, \
                 tc.tile_pool(name="p5o", bufs=3) as p5o:
                for ot in range(8):
                    pt = p5tp.tile([128, D], BF16, name="optT")
                    for dc in range(DC):
                        nc.tensor.transpose(
                            pt[:, dc * 128:(dc + 1) * 128],
                            outT[dc][:, ot * 128:(ot + 1) * 128], identb)
                    o_sb = p5o.tile([128, D], BF16, name="o_sb")
                    nc.gpsimd.tensor_copy(o_sb, pt)
                    nc.sync.dma_start(
                        out=out_own[ot * 128:(ot + 1) * 128, :], in_=o_sb)
        pX2.release()
        singles.release()

    nc.compile()
    return nc


def _get_nc():
    if "nc" not in _CACHE:
        _CACHE["nc"] = _build()
    return _CACHE["nc"]


def _pack2(w, scale):
    """[d_in, d_out] -> [d_in//256*128, 2*d_out] fp8 DoubleRow pair layout."""
    w = np.asarray(w, np.float32)
    d_in, d_out = w.shape
    nj = d_in // 256
    out = np.empty((nj * 128, 2 * d_out), np.float32)
    for j in range(nj):
        out[j * 128:(j + 1) * 128, :d_out] = w[j * 256:j * 256 + 128, :]
        out[j * 128:(j + 1) * 128, d_out:] = w[j * 256 + 128:j * 256 + 256, :]
    out = np.clip(out * scale, -224.0, 224.0)
    return np.ascontiguousarray(out.astype(f8))


def _make_in_maps(inputs):
    x = np.asarray(inputs["x"], np.float32)
    enc = np.asarray(inputs["encoder_hidden_states"], np.float32)
    scale = np.float32(1.0 / np.sqrt(HD))

    f32 = lambda a: np.ascontiguousarray(np.asarray(a, np.float32))
    g1 = np.asarray(inputs["ln1_g"], np.float64)
    b1 = np.asarray(inputs["ln1_b"], np.float64)
    g2 = np.asarray(inputs["ln2_g"], np.float64)
    b2 = np.asarray(inputs["ln2_b"], np.float64)
    sWq = np.asarray(inputs["sWq"], np.float64)
    sWk = np.asarray(inputs["sWk"], np.float64)
    sWv = np.asarray(inputs["sWv"], np.float64)
    mW1 = np.asarray(inputs["mW1"], np.float64)

    shared = dict(
        wqp=_pack2(g1[:, None] * sWq * scale, SW),
        bq=f32((b1 @ sWq + np.asarray(inputs["sbq"], np.float64)) * scale),
        wkp=_pack2(g1[:, None] * sWk, SW),
        bk=f32(b1 @ sWk + np.asarray(inputs["sbk"], np.float64)),
        wvp=_pack2(g1[:, None] * sWv, SW),
        bv=f32(b1 @ sWv + np.asarray(inputs["sbv"], np.float64)),
        wop=_pack2(np.asarray(inputs["sWo"]), SW),
        cwqp=_pack2(np.asarray(inputs["cWq"], np.float64) * scale, SW),
        cbq=f32(np.asarray(inputs["cbq"], np.float64) * scale),
        cwkp=_pack2(np.asarray(inputs["cWk"]), SW),
        cbk=f32(inputs["cbk"]),
        cwvp=_pack2(np.asarray(inputs["cWv"]), SW),
        cbv=f32(inputs["cbv"]),
        cwop=_pack2(np.asarray(inputs["cWo"]), SW),
        mw1p=_pack2(g2[:, None] * mW1, SW),
        mb1=f32(np.asarray(inputs["mb1"], np.float64) + b2 @ mW1),
        mw2p=_pack2(np.asarray(inputs["mW2"]), SW2),
        mb2x=f32(np.asarray(inputs["mb2"], np.float64) * SW2),
        resg=f32(inputs["ln1_g"]),
        resb=f32(b1 + np.asarray(inputs["sbo"], np.float64)
                 + np.asarray(inputs["cbo"], np.float64)),
    )
    kk = np.arange(128)[:, None]
    jq = np.arange(1024)[None, :]
    in_maps = []
    for c in range(8):
        b, p = c // 2, c % 2
        m = dict(shared)
        xb = x[b]
        if p == 1:
            xb = xb.reshape(T // 2, 2, D)[:, ::-1, :].reshape(T, D)
        m["x_full"] = np.ascontiguousarray(xb.astype(ml_dtypes.bfloat16))
        m["encp"] = _pack2(enc[b].T, SE)
        # key row k of a 128-chunk holds global key 128*j + kg(k)
        if p == 0:
            kg = kk
        else:
            kg = kk + 1 - 2 * (kk % 2)
        valid = (2 * (jq % 256) + p) >= (128 * (jq // 256) + kg)
        m["maskq"] = np.ascontiguousarray(
            np.where(valid, 0.0, -30000.0).astype(ml_dtypes.bfloat16))
        in_maps.append(m)
    return in_maps


def kernel(**inputs):
    in_maps = _make_in_maps(inputs)
    nc = _get_nc()
    res = run_bass_kernel_spmd(nc, in_maps, core_ids=list(range(8)))
    out = np.empty((B, T, NINP), np.float32)
    for c in range(8):
        b, p = c // 2, c % 2
        out[b, p::2] = np.asarray(res.results[c]["out_own"], np.float32)
    return out
